# revision 7
# baseline (speedup 1.0000x reference)
"""Trainium2 Bass kernel for nn_ApplyAttentionMemory.

reference:
    scores[b, l]  = sum_e query[b, e] * memory[b, l, e]
    scores        = min(scores, where(l < memory_mask[b], F32_MAX, F32_MIN))
    attention     = softmax(scores, axis=-1)                    # [B, L]
    weighted[b,e] = sum_l attention[b, l] * output_memory[b, l, e]
    returns (attention, weighted)

Sharding: data-parallel over batch. B=32 over 8 cores -> 4 batches/core.
Per core per batch:
  phase 1: stream memory[b] in [128, E] L-tiles; fused DVE
           tensor_tensor_reduce (multiply by broadcast query + free-axis
           sum) produces scores[128, 16] (l = t*128 + p).
  softmax: mask via tensor_tensor(min) with a host-precomputed lower
           bound, free-axis max (DVE) + partition all-reduce max
           (GPSIMD), exp with per-partition bias + fused row-sum (ACT),
           partition all-reduce add, reciprocal, scale.
  phase 2: stream output_memory[b]; PE matmuls with attention column as
           the stationary [128, 1] operand accumulate sum_l att*om into
           PSUM [1, E] over the 16 L-tiles.
"""

import numpy as np

F32_MAX = float(np.finfo(np.float32).max)
F32_MIN = float(np.finfo(np.float32).min)

B, L, E = 32, 2048, 1024
N_CORES = 8
BL = B // N_CORES          # batches per core
P = 128                    # SBUF partitions
T = L // P                 # L-tiles per batch
NE_HALF = E // 2           # matmul N (one PSUM bank, fp32)

_CACHE = {}


def _build_nc(do_phase2=True, do_softmax=True, att_transposed=True,
              do_qbcast=True):
    from contextlib import ExitStack

    import concourse.tile as tile
    from concourse import bacc, mybir

    f32 = mybir.dt.float32
    nc = bacc.Bacc("TRN2", target_bir_lowering=False, debug=False,
                   num_devices=N_CORES)

    mem = nc.dram_tensor("mem", [BL, L, E], f32, kind="ExternalInput").ap()
    om = nc.dram_tensor("om", [BL, L, E], f32, kind="ExternalInput").ap()
    q = nc.dram_tensor("q", [BL, E], f32, kind="ExternalInput").ap()
    lb = nc.dram_tensor("lb", [P, BL, T], f32, kind="ExternalInput").ap()
    att = nc.dram_tensor("att", [BL, L], f32, kind="ExternalOutput").ap()
    wo = nc.dram_tensor("wo", [BL, E], f32, kind="ExternalOutput").ap()

    Alu = mybir.AluOpType
    Act = mybir.ActivationFunctionType
    from concourse.bass_isa import ReduceOp

    with tile.TileContext(nc) as tc, ExitStack() as ctx:
        consts = ctx.enter_context(tc.tile_pool(name="consts", bufs=1))
        mem_pool = ctx.enter_context(tc.tile_pool(name="memp", bufs=6))
        om_pool = ctx.enter_context(tc.tile_pool(name="omp", bufs=6))
        scratch = ctx.enter_context(tc.tile_pool(name="scr", bufs=4))
        small = ctx.enter_context(tc.tile_pool(name="small", bufs=24))
        psum_pool = ctx.enter_context(
            tc.tile_pool(name="ps", bufs=4, space="PSUM"))
        wo_pool = ctx.enter_context(tc.tile_pool(name="wop", bufs=4))

        lb_sb = consts.tile([P, BL, T], f32)
        nc.sync.dma_start(out=lb_sb, in_=lb)
        q_sb = consts.tile([P, BL, E], f32)
        for b in range(BL):
            if do_qbcast:
                nc.gpsimd.dma_start(
                    out=q_sb[:, b, :],
                    in_=q[b:b + 1, :].partition_broadcast(P)[:, 0, :])
            else:
                nc.vector.memset(q_sb[:, b, :], 1.0)

        for b in range(BL):
            scores = small.tile([P, T], f32, tag="scores")
            for t in range(T):
                m = mem_pool.tile([P, E], f32, tag="m")
                nc.sync.dma_start(out=m, in_=mem[b, t * P:(t + 1) * P, :])
                scr = scratch.tile([P, E], f32, tag="scr")
                nc.vector.affine_mul_reduce(
                    out=scr, accum_out=scores[:, t:t + 1],
                    in0=m, in1=q_sb[:, b, :], scale=1.0, bias=0.0)

            if do_softmax:
                sm = small.tile([P, T], f32, tag="sm")
                nc.vector.tensor_tensor(out=sm, in0=scores,
                                        in1=lb_sb[:, b, :], op=Alu.min)
                pmax = small.tile([P, 1], f32, tag="pmax")
                nc.vector.tensor_reduce(out=pmax, in_=sm,
                                        axis=mybir.AxisListType.X, op=Alu.max)
                gmax = small.tile([P, 1], f32, tag="gmax")
                nc.gpsimd.partition_all_reduce(gmax, pmax, P, ReduceOp.max)
                negmax = small.tile([P, 1], f32, tag="negmax")
                nc.vector.tensor_scalar_mul(negmax, gmax, -1.0)

                p_t = small.tile([P, T], f32, tag="pt")
                sum_p = small.tile([P, 1], f32, tag="sump")
                nc.scalar.activation(out=p_t, in_=sm, func=Act.Exp,
                                     bias=negmax, accum_out=sum_p)
                gsum = small.tile([P, 1], f32, tag="gsum")
                nc.gpsimd.partition_all_reduce(gsum, sum_p, P, ReduceOp.add)
                rinv = small.tile([P, 1], f32, tag="rinv")
                nc.vector.reciprocal(rinv, gsum)
                att_t = small.tile([P, T], f32, tag="att")
                nc.vector.tensor_scalar_mul(att_t, p_t, rinv)
            else:
                att_t = small.tile([P, T], f32, tag="att")
                nc.vector.tensor_scalar_mul(att_t, scores, 1.0)
            if att_transposed:
                nc.gpsimd.dma_start(out=att[b].rearrange("(t p) -> p t", p=P),
                                    in_=att_t)
            else:
                nc.sync.dma_start(out=att[b].rearrange("(p t) -> p t", p=P),
                                  in_=att_t)

            w = wo_pool.tile([1, E], f32, tag="w")
            if do_phase2:
                ps0 = psum_pool.tile([1, NE_HALF], f32, tag="ps0")
                ps1 = psum_pool.tile([1, NE_HALF], f32, tag="ps1")
                for t in range(T):
                    o = om_pool.tile([P, E], f32, tag="o")
                    nc.sync.dma_start(out=o, in_=om[b, t * P:(t + 1) * P, :])
                    lhsT = att_t[:, t:t + 1]
                    nc.tensor.matmul(ps0, lhsT=lhsT, rhs=o[:, 0:NE_HALF],
                                     start=(t == 0), stop=(t == T - 1))
                    nc.tensor.matmul(ps1, lhsT=lhsT, rhs=o[:, NE_HALF:E],
                                     start=(t == 0), stop=(t == T - 1))
                nc.vector.tensor_copy(w[:, 0:NE_HALF], ps0)
                nc.vector.tensor_copy(w[:, NE_HALF:E], ps1)
            else:
                nc.vector.memset(w, 0.0)
            nc.sync.dma_start(out=wo[b:b + 1, :], in_=w)

    nc.compile()
    return nc


def _get_nc():
    if "nc" not in _CACHE:
        _CACHE["nc"] = _build_nc()
    return _CACHE["nc"]


def kernel(memory, output_memory, query, memory_mask, maxlen):
    from concourse.bass_utils import run_bass_kernel_spmd

    memory = np.ascontiguousarray(np.asarray(memory), dtype=np.float32)
    output_memory = np.ascontiguousarray(np.asarray(output_memory),
                                         dtype=np.float32)
    query = np.ascontiguousarray(np.asarray(query), dtype=np.float32)
    memory_mask = np.asarray(memory_mask).astype(np.int64)
    maxlen = int(maxlen)
    assert memory.shape == (B, L, E) and query.shape == (B, E)
    assert maxlen == L

    kept = np.arange(L)[None, :] < memory_mask[:, None]        # [B, L]
    lb_full = np.where(kept, F32_MAX, F32_MIN).astype(np.float32)
    # device layout: lb_dev[core][p, b_local, t] = lb_full[core*BL+b, t*P+p]
    lb_dev = lb_full.reshape(N_CORES, BL, T, P).transpose(0, 3, 1, 2)

    in_maps = [{
        "mem": memory[c * BL:(c + 1) * BL],
        "om": output_memory[c * BL:(c + 1) * BL],
        "q": query[c * BL:(c + 1) * BL],
        "lb": np.ascontiguousarray(lb_dev[c]),
    } for c in range(N_CORES)]

    res = run_bass_kernel_spmd(_get_nc(), in_maps,
                               core_ids=list(range(N_CORES)))
    att = np.concatenate([res.results[c]["att"] for c in range(N_CORES)], 0)
    wo = np.concatenate([res.results[c]["wo"] for c in range(N_CORES)], 0)
    return att.astype(np.float32), wo.astype(np.float32)


# revision 10
# speedup vs baseline: 1.2238x; 1.2238x over previous
"""Trainium2 Bass kernel for nn_ApplyAttentionMemory.

reference:
    scores[b, l]  = sum_e query[b, e] * memory[b, l, e]
    scores        = min(scores, where(l < memory_mask[b], F32_MAX, F32_MIN))
    attention     = softmax(scores, axis=-1)                    # [B, L]
    weighted[b,e] = sum_l attention[b, l] * output_memory[b, l, e]
    returns (attention, weighted)

Sharding: data-parallel over batch. B=32 over 8 cores -> 4 batches/core.
Per core per batch:
  phase 1: stream memory[b] in [128, E] L-tiles; fused DVE
           tensor_tensor_reduce (multiply by broadcast query + free-axis
           sum) produces scores[128, 16] (l = t*128 + p).
  softmax: mask via tensor_tensor(min) with a host-precomputed lower
           bound, free-axis max (DVE) + partition all-reduce max
           (GPSIMD), exp with per-partition bias + fused row-sum (ACT),
           partition all-reduce add, reciprocal, scale.
  phase 2: stream output_memory[b]; PE matmuls with attention column as
           the stationary [128, 1] operand accumulate sum_l att*om into
           PSUM [1, E] over the 16 L-tiles.
"""

import numpy as np

F32_MAX = float(np.finfo(np.float32).max)
F32_MIN = float(np.finfo(np.float32).min)

B, L, E = 32, 2048, 1024
N_CORES = 8
BL = B // N_CORES          # batches per core
P = 128                    # SBUF partitions
T = L // P                 # L-tiles per batch
NE_HALF = E // 2           # matmul N (one PSUM bank, fp32)

_CACHE = {}


def _lb_layout(lb_full):
    """[B, L] lower bound -> per-core device layout [8][P, BL, T]."""
    return lb_full.reshape(N_CORES, BL, T, P).transpose(0, 3, 1, 2)


def _build_nc(do_phase2=True, do_softmax=True, att_transposed=True,
              do_qbcast=True):
    from contextlib import ExitStack

    import concourse.tile as tile
    from concourse import bacc, mybir

    f32 = mybir.dt.float32
    nc = bacc.Bacc("TRN2", target_bir_lowering=False, debug=False,
                   num_devices=N_CORES)

    mem = nc.dram_tensor("mem", [BL, L, E], f32, kind="ExternalInput").ap()
    om = nc.dram_tensor("om", [BL, L, E], f32, kind="ExternalInput").ap()
    q = nc.dram_tensor("q", [BL, E], f32, kind="ExternalInput").ap()
    lb = nc.dram_tensor("lb", [P, BL, T], f32, kind="ExternalInput").ap()
    att = nc.dram_tensor("att", [BL, L], f32, kind="ExternalOutput").ap()
    wo = nc.dram_tensor("wo", [BL, E], f32, kind="ExternalOutput").ap()

    Alu = mybir.AluOpType
    Act = mybir.ActivationFunctionType
    from concourse.bass_isa import ReduceOp

    with tile.TileContext(nc) as tc, ExitStack() as ctx:
        consts = ctx.enter_context(tc.tile_pool(name="consts", bufs=1))
        mem_pool = ctx.enter_context(tc.tile_pool(name="memp", bufs=10))
        om_pool = ctx.enter_context(tc.tile_pool(name="omp", bufs=10))
        scratch = ctx.enter_context(tc.tile_pool(name="scr", bufs=4))
        small = ctx.enter_context(tc.tile_pool(name="small", bufs=24))
        psum_pool = ctx.enter_context(
            tc.tile_pool(name="ps", bufs=4, space="PSUM"))
        wo_pool = ctx.enter_context(tc.tile_pool(name="wop", bufs=4))

        lb_sb = consts.tile([P, BL, T], f32)
        nc.sync.dma_start(out=lb_sb, in_=lb)
        q_sb = consts.tile([P, BL, E], f32)
        for b in range(BL):
            if do_qbcast:
                nc.gpsimd.dma_start(
                    out=q_sb[:, b, :],
                    in_=q[b:b + 1, :].partition_broadcast(P)[:, 0, :])
            else:
                nc.vector.memset(q_sb[:, b, :], 1.0)

        for b in range(BL):
            scores = small.tile([P, T], f32, tag="scores")
            for t in range(T):
                m = mem_pool.tile([P, E], f32, tag="m")
                nc.sync.dma_start(out=m, in_=mem[b, t * P:(t + 1) * P, :])
                scr = scratch.tile([P, E], f32, tag="scr")
                nc.vector.affine_mul_reduce(
                    out=scr, accum_out=scores[:, t:t + 1],
                    in0=m, in1=q_sb[:, b, :], scale=1.0, bias=0.0)

            if do_softmax:
                sm = small.tile([P, T], f32, tag="sm")
                nc.vector.tensor_tensor(out=sm, in0=scores,
                                        in1=lb_sb[:, b, :], op=Alu.min)
                pmax = small.tile([P, 1], f32, tag="pmax")
                nc.vector.tensor_reduce(out=pmax, in_=sm,
                                        axis=mybir.AxisListType.X, op=Alu.max)
                gmax = small.tile([P, 1], f32, tag="gmax")
                nc.gpsimd.partition_all_reduce(gmax, pmax, P, ReduceOp.max)
                negmax = small.tile([P, 1], f32, tag="negmax")
                nc.vector.tensor_scalar_mul(negmax, gmax, -1.0)

                p_t = small.tile([P, T], f32, tag="pt")
                sum_p = small.tile([P, 1], f32, tag="sump")
                nc.scalar.activation(out=p_t, in_=sm, func=Act.Exp,
                                     bias=negmax, accum_out=sum_p)
                gsum = small.tile([P, 1], f32, tag="gsum")
                nc.gpsimd.partition_all_reduce(gsum, sum_p, P, ReduceOp.add)
                rinv = small.tile([P, 1], f32, tag="rinv")
                nc.vector.reciprocal(rinv, gsum)
                att_t = small.tile([P, T], f32, tag="att")
                nc.vector.tensor_scalar_mul(att_t, p_t, rinv)
            else:
                att_t = small.tile([P, T], f32, tag="att")
                nc.vector.tensor_scalar_mul(att_t, scores, 1.0)
            if att_transposed:
                nc.gpsimd.dma_start(out=att[b].rearrange("(t p) -> p t", p=P),
                                    in_=att_t)
            else:
                nc.sync.dma_start(out=att[b].rearrange("(p t) -> p t", p=P),
                                  in_=att_t)

            w = wo_pool.tile([1, E], f32, tag="w")
            if do_phase2:
                ps0 = psum_pool.tile([1, NE_HALF], f32, tag="ps0")
                ps1 = psum_pool.tile([1, NE_HALF], f32, tag="ps1")
                for t in range(T):
                    o = om_pool.tile([P, E], f32, tag="o")
                    nc.sync.dma_start(out=o, in_=om[b, t * P:(t + 1) * P, :])
                    lhsT = att_t[:, t:t + 1]
                    nc.tensor.matmul(ps0, lhsT=lhsT, rhs=o[:, 0:NE_HALF],
                                     start=(t == 0), stop=(t == T - 1))
                    nc.tensor.matmul(ps1, lhsT=lhsT, rhs=o[:, NE_HALF:E],
                                     start=(t == 0), stop=(t == T - 1))
                nc.vector.tensor_copy(w[:, 0:NE_HALF], ps0)
                nc.vector.tensor_copy(w[:, NE_HALF:E], ps1)
            else:
                nc.vector.memset(w, 0.0)
            nc.sync.dma_start(out=wo[b:b + 1, :], in_=w)

    nc.compile()
    return nc


def _get_nc():
    if "nc" not in _CACHE:
        _CACHE["nc"] = _build_nc()
    return _CACHE["nc"]


def kernel(memory, output_memory, query, memory_mask, maxlen):
    from concourse.bass_utils import run_bass_kernel_spmd

    memory = np.ascontiguousarray(np.asarray(memory), dtype=np.float32)
    output_memory = np.ascontiguousarray(np.asarray(output_memory),
                                         dtype=np.float32)
    query = np.ascontiguousarray(np.asarray(query), dtype=np.float32)
    memory_mask = np.asarray(memory_mask).astype(np.int64)
    maxlen = int(maxlen)
    assert memory.shape == (B, L, E) and query.shape == (B, E)
    assert maxlen == L

    kept = np.arange(L)[None, :] < memory_mask[:, None]        # [B, L]
    lb_full = np.where(kept, F32_MAX, F32_MIN).astype(np.float32)
    lb_dev = _lb_layout(lb_full)

    in_maps = [{
        "mem": memory[c * BL:(c + 1) * BL],
        "om": output_memory[c * BL:(c + 1) * BL],
        "q": query[c * BL:(c + 1) * BL],
        "lb": np.ascontiguousarray(lb_dev[c]),
    } for c in range(N_CORES)]

    res = run_bass_kernel_spmd(_get_nc(), in_maps,
                               core_ids=list(range(N_CORES)))
    att = np.concatenate([res.results[c]["att"] for c in range(N_CORES)], 0)
    wo = np.concatenate([res.results[c]["wo"] for c in range(N_CORES)], 0)
    return att.astype(np.float32), wo.astype(np.float32)


# revision 16
# speedup vs baseline: 1.2618x; 1.0310x over previous
"""Trainium2 Bass kernel for nn_ApplyAttentionMemory.

reference:
    scores[b, l]  = sum_e query[b, e] * memory[b, l, e]
    scores        = min(scores, where(l < memory_mask[b], F32_MAX, F32_MIN))
    attention     = softmax(scores, axis=-1)                    # [B, L]
    weighted[b,e] = sum_l attention[b, l] * output_memory[b, l, e]
    returns (attention, weighted)

Sharding: data-parallel over batch. B=32 over 8 cores -> 4 batches/core.

Layout: L is tiled into big tiles of RPP*128 rows; partition p of a big
tile holds RPP consecutive rows (l = 128*RPP*t + RPP*p + r), giving
RPP*4KiB contiguous DRAM per partition per DMA (large descriptors).

Per core per batch:
  phase 1: stream memory[b]; fused DVE affine_mul_reduce (multiply by
           partition-broadcast query + free-axis sum) gives
           scores[128, 16] (col c = RPP*t + r).
  softmax: mask via tensor_tensor(min) with host-precomputed lower
           bound, free-axis max (DVE) + partition all-reduce max
           (GPSIMD), exp with per-partition bias + fused row-sum (ACT),
           partition all-reduce add, reciprocal, scale.
  phase 2: stream output_memory[b]; PE f32 matmuls with the attention
           column as stationary [128, 1] accumulate sum_l att*om into
           PSUM [1, E] across the L tiles.
"""

import numpy as np

F32_MAX = float(np.finfo(np.float32).max)
F32_MIN = float(np.finfo(np.float32).min)

B, L, E = 32, 2048, 1024
N_CORES = 8
BL = B // N_CORES          # batches per core
P = 128                    # SBUF partitions
RPP = 2                    # L rows per partition per big tile
TB = L // (P * RPP)        # big tiles per batch
NCOL = L // P              # score columns (= 16)
NE_HALF = E // 2           # matmul N (one PSUM bank, fp32)

_CACHE = {}


def _lb_layout(lb_full):
    """[B, L] lower bound -> per-core device layout [8][P, BL, NCOL]."""
    # col c = RPP*t + r  <->  l = P*RPP*t + RPP*p + r
    x = lb_full.reshape(N_CORES, BL, TB, P, RPP)
    return x.transpose(0, 3, 1, 2, 4).reshape(N_CORES, P, BL, NCOL)


def _lb_layout_core(lb_core):
    """[BL, L] lower bound -> single-core device layout [P, BL, NCOL]."""
    x = lb_core.reshape(BL, TB, P, RPP)
    return x.transpose(2, 0, 1, 3).reshape(P, BL, NCOL)


def _build_nc(mem_bufs=7, om_bufs=7):
    from contextlib import ExitStack

    import concourse.tile as tile
    from concourse import bacc, mybir

    f32 = mybir.dt.float32
    nc = bacc.Bacc("TRN2", target_bir_lowering=False, debug=False,
                   num_devices=N_CORES)

    mem = nc.dram_tensor("mem", [BL, L, E], f32, kind="ExternalInput").ap()
    om = nc.dram_tensor("om", [BL, L, E], f32, kind="ExternalInput").ap()
    q = nc.dram_tensor("q", [BL, E], f32, kind="ExternalInput").ap()
    lb = nc.dram_tensor("lb", [P, BL, NCOL], f32, kind="ExternalInput").ap()
    att = nc.dram_tensor("att", [BL, L], f32, kind="ExternalOutput").ap()
    wo = nc.dram_tensor("wo", [BL, E], f32, kind="ExternalOutput").ap()

    Alu = mybir.AluOpType
    Act = mybir.ActivationFunctionType
    from concourse.bass_isa import ReduceOp

    FB = RPP * E           # free bytes per big tile row-group
    with tile.TileContext(nc) as tc, ExitStack() as ctx:
        consts = ctx.enter_context(tc.tile_pool(name="consts", bufs=1))
        mem_pool = ctx.enter_context(tc.tile_pool(name="memp", bufs=mem_bufs))
        om_pool = ctx.enter_context(tc.tile_pool(name="omp", bufs=om_bufs))
        scratch = ctx.enter_context(tc.tile_pool(name="scr", bufs=4))
        small = ctx.enter_context(tc.tile_pool(name="small", bufs=24))
        psum_pool = ctx.enter_context(
            tc.tile_pool(name="ps", bufs=4, space="PSUM"))
        wo_pool = ctx.enter_context(tc.tile_pool(name="wop", bufs=2))

        lb_sb = consts.tile([P, BL, NCOL], f32)
        nc.sync.dma_start(out=lb_sb, in_=lb)
        q_sb = consts.tile([P, BL, E], f32)
        for b in range(BL):
            nc.gpsimd.dma_start(
                out=q_sb[:, b, :],
                in_=q[b:b + 1, :].partition_broadcast(P)[:, 0, :])

        for b in range(BL):
            scores = small.tile([P, NCOL], f32, tag="scores")
            for t in range(TB):
                m = mem_pool.tile([P, FB], f32, tag="m")
                nc.sync.dma_start(
                    out=m,
                    in_=mem[b, t * P * RPP:(t + 1) * P * RPP, :].rearrange(
                        "(p r) e -> p (r e)", p=P))
                for r in range(RPP):
                    scr = scratch.tile([P, E], f32, tag="scr")
                    nc.vector.affine_mul_reduce(
                        out=scr, accum_out=scores[:, RPP * t + r:RPP * t + r + 1],
                        in0=m[:, r * E:(r + 1) * E], in1=q_sb[:, b, :],
                        scale=1.0, bias=0.0)

            sm = small.tile([P, NCOL], f32, tag="sm")
            nc.vector.tensor_tensor(out=sm, in0=scores,
                                    in1=lb_sb[:, b, :], op=Alu.min)
            pmax = small.tile([P, 1], f32, tag="pmax")
            nc.vector.tensor_reduce(out=pmax, in_=sm,
                                    axis=mybir.AxisListType.X, op=Alu.max)
            gmax = small.tile([P, 1], f32, tag="gmax")
            nc.gpsimd.partition_all_reduce(gmax, pmax, P, ReduceOp.max)
            negmax = small.tile([P, 1], f32, tag="negmax")
            nc.vector.tensor_scalar_mul(negmax, gmax, -1.0)

            p_t = small.tile([P, NCOL], f32, tag="pt")
            sum_p = small.tile([P, 1], f32, tag="sump")
            nc.scalar.activation(out=p_t, in_=sm, func=Act.Exp,
                                 bias=negmax, accum_out=sum_p)
            gsum = small.tile([P, 1], f32, tag="gsum")
            nc.gpsimd.partition_all_reduce(gsum, sum_p, P, ReduceOp.add)
            rinv = small.tile([P, 1], f32, tag="rinv")
            nc.vector.reciprocal(rinv, gsum)
            att_t = small.tile([P, NCOL], f32, tag="att")
            nc.vector.tensor_scalar_mul(att_t, p_t, rinv)
            nc.gpsimd.dma_start(
                out=att[b].rearrange("(t p r) -> p t r", p=P, r=RPP),
                in_=att_t.rearrange("p (t r) -> p t r", r=RPP))

            ps0 = psum_pool.tile([1, NE_HALF], f32, tag="ps0")
            ps1 = psum_pool.tile([1, NE_HALF], f32, tag="ps1")
            for t in range(TB):
                o = om_pool.tile([P, FB], f32, tag="o")
                nc.sync.dma_start(
                    out=o,
                    in_=om[b, t * P * RPP:(t + 1) * P * RPP, :].rearrange(
                        "(p r) e -> p (r e)", p=P))
                for r in range(RPP):
                    c = RPP * t + r
                    lhsT = att_t[:, c:c + 1]
                    first = (t == 0 and r == 0)
                    last = (t == TB - 1 and r == RPP - 1)
                    nc.tensor.matmul(ps0, lhsT=lhsT,
                                     rhs=o[:, r * E:r * E + NE_HALF],
                                     start=first, stop=last)
                    nc.tensor.matmul(ps1, lhsT=lhsT,
                                     rhs=o[:, r * E + NE_HALF:(r + 1) * E],
                                     start=first, stop=last)
            w = wo_pool.tile([1, E], f32, tag="w")
            nc.vector.tensor_copy(w[:, 0:NE_HALF], ps0)
            nc.vector.tensor_copy(w[:, NE_HALF:E], ps1)
            nc.sync.dma_start(out=wo[b:b + 1, :], in_=w)

    nc.compile()
    return nc


def _get_nc():
    if "nc" not in _CACHE:
        _CACHE["nc"] = _build_nc()
    return _CACHE["nc"]


def kernel(memory, output_memory, query, memory_mask, maxlen):
    from concourse.bass_utils import run_bass_kernel_spmd

    memory = np.ascontiguousarray(np.asarray(memory), dtype=np.float32)
    output_memory = np.ascontiguousarray(np.asarray(output_memory),
                                         dtype=np.float32)
    query = np.ascontiguousarray(np.asarray(query), dtype=np.float32)
    memory_mask = np.asarray(memory_mask).astype(np.int64)
    maxlen = int(maxlen)
    assert memory.shape == (B, L, E) and query.shape == (B, E)
    assert maxlen == L

    kept = np.arange(L)[None, :] < memory_mask[:, None]        # [B, L]
    lb_full = np.where(kept, F32_MAX, F32_MIN).astype(np.float32)
    lb_dev = _lb_layout(lb_full)

    in_maps = [{
        "mem": memory[c * BL:(c + 1) * BL],
        "om": output_memory[c * BL:(c + 1) * BL],
        "q": query[c * BL:(c + 1) * BL],
        "lb": np.ascontiguousarray(lb_dev[c]),
    } for c in range(N_CORES)]

    res = run_bass_kernel_spmd(_get_nc(), in_maps,
                               core_ids=list(range(N_CORES)))
    att = np.concatenate([res.results[c]["att"] for c in range(N_CORES)], 0)
    wo = np.concatenate([res.results[c]["wo"] for c in range(N_CORES)], 0)
    return att.astype(np.float32), wo.astype(np.float32)


# revision 17
# speedup vs baseline: 1.3596x; 1.0775x over previous
"""Trainium2 Bass kernel for nn_ApplyAttentionMemory.

reference:
    scores[b, l]  = sum_e query[b, e] * memory[b, l, e]
    scores        = min(scores, where(l < memory_mask[b], F32_MAX, F32_MIN))
    attention     = softmax(scores, axis=-1)                    # [B, L]
    weighted[b,e] = sum_l attention[b, l] * output_memory[b, l, e]
    returns (attention, weighted)

Sharding: data-parallel over batch. B=32 over 8 cores -> 4 batches/core.

Layout: L is tiled into big tiles of RPP*128 rows; partition p of a big
tile holds RPP consecutive rows (l = 128*RPP*t + RPP*p + r), giving
RPP*4KiB contiguous DRAM per partition per DMA (large descriptors).

Per core per batch:
  phase 1: stream memory[b]; fused DVE affine_mul_reduce (multiply by
           partition-broadcast query + free-axis sum) gives
           scores[128, 16] (col c = RPP*t + r).
  softmax: mask via tensor_tensor(min) with host-precomputed lower
           bound, free-axis max (DVE) + partition all-reduce max
           (GPSIMD), exp with per-partition bias + fused row-sum (ACT),
           partition all-reduce add, reciprocal, scale.
  phase 2: stream output_memory[b]; PE f32 matmuls with the attention
           column as stationary [128, 1] accumulate sum_l att*om into
           PSUM [1, E] across the L tiles.
"""

import numpy as np

F32_MAX = float(np.finfo(np.float32).max)
F32_MIN = float(np.finfo(np.float32).min)

B, L, E = 32, 2048, 1024
N_CORES = 8
BL = B // N_CORES          # batches per core
P = 128                    # SBUF partitions
RPP = 2                    # L rows per partition per big tile
TB = L // (P * RPP)        # big tiles per batch
NCOL = L // P              # score columns (= 16)
NE_HALF = E // 2           # matmul N (one PSUM bank, fp32)

_CACHE = {}


def _lb_layout(lb_full):
    """[B, L] lower bound -> per-core device layout [8][P, BL, NCOL]."""
    # col c = RPP*t + r  <->  l = P*RPP*t + RPP*p + r
    x = lb_full.reshape(N_CORES, BL, TB, P, RPP)
    return x.transpose(0, 3, 1, 2, 4).reshape(N_CORES, P, BL, NCOL)


def _lb_layout_core(lb_core):
    """[BL, L] lower bound -> single-core device layout [P, BL, NCOL]."""
    x = lb_core.reshape(BL, TB, P, RPP)
    return x.transpose(2, 0, 1, 3).reshape(P, BL, NCOL)


def _build_nc(mem_bufs=7, om_bufs=7):
    from contextlib import ExitStack

    import concourse.tile as tile
    from concourse import bacc, mybir

    f32 = mybir.dt.float32
    nc = bacc.Bacc("TRN2", target_bir_lowering=False, debug=False,
                   num_devices=N_CORES)

    mem = nc.dram_tensor("mem", [BL, L, E], f32, kind="ExternalInput").ap()
    om = nc.dram_tensor("om", [BL, L, E], f32, kind="ExternalInput").ap()
    q = nc.dram_tensor("q", [BL, E], f32, kind="ExternalInput").ap()
    lb = nc.dram_tensor("lb", [P, BL, NCOL], f32, kind="ExternalInput").ap()
    att = nc.dram_tensor("att", [BL, L], f32, kind="ExternalOutput").ap()
    wo = nc.dram_tensor("wo", [BL, E], f32, kind="ExternalOutput").ap()

    Alu = mybir.AluOpType
    Act = mybir.ActivationFunctionType
    from concourse.bass_isa import ReduceOp

    FB = RPP * E           # free bytes per big tile row-group
    with tile.TileContext(nc) as tc, ExitStack() as ctx:
        consts = ctx.enter_context(tc.tile_pool(name="consts", bufs=1))
        mem_pool = ctx.enter_context(tc.tile_pool(name="memp", bufs=mem_bufs))
        om_pool = ctx.enter_context(tc.tile_pool(name="omp", bufs=om_bufs))
        scratch = ctx.enter_context(tc.tile_pool(name="scr", bufs=4))
        small = ctx.enter_context(tc.tile_pool(name="small", bufs=24))
        psum_pool = ctx.enter_context(
            tc.tile_pool(name="ps", bufs=4, space="PSUM"))
        wo_pool = ctx.enter_context(tc.tile_pool(name="wop", bufs=2))

        lb_sb = consts.tile([P, BL, NCOL], f32)
        nc.sync.dma_start(out=lb_sb, in_=lb)
        q_sb = consts.tile([P, BL, E], f32)
        for b in range(BL):
            nc.gpsimd.dma_start(
                out=q_sb[:, b, :],
                in_=q[b:b + 1, :].partition_broadcast(P)[:, 0, :])

        for b in range(BL):
            scores = small.tile([P, NCOL], f32, tag="scores")
            for t in range(TB):
                m = mem_pool.tile([P, FB], f32, tag="m")
                nc.sync.dma_start(
                    out=m,
                    in_=mem[b, t * P * RPP:(t + 1) * P * RPP, :].rearrange(
                        "(p r) e -> p (r e)", p=P))
                for r in range(RPP):
                    scr = scratch.tile([P, E], f32, tag="scr")
                    nc.vector.affine_mul_reduce(
                        out=scr, accum_out=scores[:, RPP * t + r:RPP * t + r + 1],
                        in0=m[:, r * E:(r + 1) * E], in1=q_sb[:, b, :],
                        scale=1.0, bias=0.0)

            sm = small.tile([P, NCOL], f32, tag="sm")
            nc.vector.tensor_tensor(out=sm, in0=scores,
                                    in1=lb_sb[:, b, :], op=Alu.min)
            pmax = small.tile([P, 1], f32, tag="pmax")
            nc.vector.tensor_reduce(out=pmax, in_=sm,
                                    axis=mybir.AxisListType.X, op=Alu.max)
            gmax = small.tile([P, 1], f32, tag="gmax")
            nc.gpsimd.partition_all_reduce(gmax, pmax, P, ReduceOp.max)
            negmax = small.tile([P, 1], f32, tag="negmax")
            nc.vector.tensor_scalar_mul(negmax, gmax, -1.0)

            p_t = small.tile([P, NCOL], f32, tag="pt")
            sum_p = small.tile([P, 1], f32, tag="sump")
            nc.scalar.activation(out=p_t, in_=sm, func=Act.Exp,
                                 bias=negmax, accum_out=sum_p)
            gsum = small.tile([P, 1], f32, tag="gsum")
            nc.gpsimd.partition_all_reduce(gsum, sum_p, P, ReduceOp.add)
            rinv = small.tile([P, 1], f32, tag="rinv")
            nc.vector.reciprocal(rinv, gsum)
            att_t = small.tile([P, NCOL], f32, tag="att")
            nc.vector.tensor_scalar_mul(att_t, p_t, rinv)
            nc.gpsimd.dma_start(
                out=att[b].rearrange("(t p r) -> p t r", p=P, r=RPP),
                in_=att_t.rearrange("p (t r) -> p t r", r=RPP))

            # matmuls use the unnormalized exp(p_t); 1/sum is folded into
            # the PSUM->SBUF copy so they don't wait on the sum/reciprocal
            ps0 = psum_pool.tile([1, NE_HALF], f32, tag="ps0")
            ps1 = psum_pool.tile([1, NE_HALF], f32, tag="ps1")
            for t in range(TB):
                o = om_pool.tile([P, FB], f32, tag="o")
                nc.scalar.dma_start(
                    out=o,
                    in_=om[b, t * P * RPP:(t + 1) * P * RPP, :].rearrange(
                        "(p r) e -> p (r e)", p=P))
                for r in range(RPP):
                    c = RPP * t + r
                    lhsT = p_t[:, c:c + 1]
                    first = (t == 0 and r == 0)
                    last = (t == TB - 1 and r == RPP - 1)
                    nc.tensor.matmul(ps0, lhsT=lhsT,
                                     rhs=o[:, r * E:r * E + NE_HALF],
                                     start=first, stop=last)
                    nc.tensor.matmul(ps1, lhsT=lhsT,
                                     rhs=o[:, r * E + NE_HALF:(r + 1) * E],
                                     start=first, stop=last)
            w = wo_pool.tile([1, E], f32, tag="w")
            nc.vector.tensor_scalar_mul(w[:, 0:NE_HALF], ps0, rinv[0:1, :])
            nc.vector.tensor_scalar_mul(w[:, NE_HALF:E], ps1, rinv[0:1, :])
            nc.scalar.dma_start(out=wo[b:b + 1, :], in_=w)

    nc.compile()
    return nc


def _get_nc():
    if "nc" not in _CACHE:
        _CACHE["nc"] = _build_nc()
    return _CACHE["nc"]


def kernel(memory, output_memory, query, memory_mask, maxlen):
    from concourse.bass_utils import run_bass_kernel_spmd

    memory = np.ascontiguousarray(np.asarray(memory), dtype=np.float32)
    output_memory = np.ascontiguousarray(np.asarray(output_memory),
                                         dtype=np.float32)
    query = np.ascontiguousarray(np.asarray(query), dtype=np.float32)
    memory_mask = np.asarray(memory_mask).astype(np.int64)
    maxlen = int(maxlen)
    assert memory.shape == (B, L, E) and query.shape == (B, E)
    assert maxlen == L

    kept = np.arange(L)[None, :] < memory_mask[:, None]        # [B, L]
    lb_full = np.where(kept, F32_MAX, F32_MIN).astype(np.float32)
    lb_dev = _lb_layout(lb_full)

    in_maps = [{
        "mem": memory[c * BL:(c + 1) * BL],
        "om": output_memory[c * BL:(c + 1) * BL],
        "q": query[c * BL:(c + 1) * BL],
        "lb": np.ascontiguousarray(lb_dev[c]),
    } for c in range(N_CORES)]

    res = run_bass_kernel_spmd(_get_nc(), in_maps,
                               core_ids=list(range(N_CORES)))
    att = np.concatenate([res.results[c]["att"] for c in range(N_CORES)], 0)
    wo = np.concatenate([res.results[c]["wo"] for c in range(N_CORES)], 0)
    return att.astype(np.float32), wo.astype(np.float32)


# revision 18
# speedup vs baseline: 1.3856x; 1.0191x over previous
"""Trainium2 Bass kernel for nn_ApplyAttentionMemory.

reference:
    scores[b, l]  = sum_e query[b, e] * memory[b, l, e]
    scores        = min(scores, where(l < memory_mask[b], F32_MAX, F32_MIN))
    attention     = softmax(scores, axis=-1)                    # [B, L]
    weighted[b,e] = sum_l attention[b, l] * output_memory[b, l, e]
    returns (attention, weighted)

Sharding: data-parallel over batch. B=32 over 8 cores -> 4 batches/core.

Layout: L is tiled into big tiles of RPP*128 rows; partition p of a big
tile holds RPP consecutive rows (l = 128*RPP*t + RPP*p + r), giving
RPP*4KiB contiguous DRAM per partition per DMA (large descriptors).

Per core per batch:
  phase 1: stream memory[b]; fused DVE affine_mul_reduce (multiply by
           partition-broadcast query + free-axis sum) gives
           scores[128, 16] (col c = RPP*t + r).
  softmax: mask via tensor_tensor(min) with host-precomputed lower
           bound, free-axis max (DVE) + partition all-reduce max
           (GPSIMD), exp with per-partition bias + fused row-sum (ACT),
           partition all-reduce add, reciprocal, scale.
  phase 2: stream output_memory[b]; PE f32 matmuls with the attention
           column as stationary [128, 1] accumulate sum_l att*om into
           PSUM [1, E] across the L tiles.
"""

import numpy as np

F32_MAX = float(np.finfo(np.float32).max)
F32_MIN = float(np.finfo(np.float32).min)

B, L, E = 32, 2048, 1024
N_CORES = 8
BL = B // N_CORES          # batches per core
P = 128                    # SBUF partitions
RPP = 2                    # L rows per partition per big tile
TB = L // (P * RPP)        # big tiles per batch
NCOL = L // P              # score columns (= 16)
NE_HALF = E // 2           # matmul N (one PSUM bank, fp32)

_CACHE = {}


def _lb_layout(lb_full):
    """[B, L] lower bound -> per-core device layout [8][P, BL, NCOL]."""
    # col c = RPP*t + r  <->  l = P*RPP*t + RPP*p + r
    x = lb_full.reshape(N_CORES, BL, TB, P, RPP)
    return x.transpose(0, 3, 1, 2, 4).reshape(N_CORES, P, BL, NCOL)


def _lb_layout_core(lb_core):
    """[BL, L] lower bound -> single-core device layout [P, BL, NCOL]."""
    x = lb_core.reshape(BL, TB, P, RPP)
    return x.transpose(2, 0, 1, 3).reshape(P, BL, NCOL)


def _build_nc(mem_bufs=7, om_bufs=7):
    from contextlib import ExitStack

    import concourse.tile as tile
    from concourse import bacc, mybir

    f32 = mybir.dt.float32
    nc = bacc.Bacc("TRN2", target_bir_lowering=False, debug=False,
                   num_devices=N_CORES)

    mem = nc.dram_tensor("mem", [BL, L, E], f32, kind="ExternalInput").ap()
    om = nc.dram_tensor("om", [BL, L, E], f32, kind="ExternalInput").ap()
    q = nc.dram_tensor("q", [BL, E], f32, kind="ExternalInput").ap()
    lb = nc.dram_tensor("lb", [P, BL, NCOL], f32, kind="ExternalInput").ap()
    att = nc.dram_tensor("att", [BL, L], f32, kind="ExternalOutput").ap()
    wo = nc.dram_tensor("wo", [BL, E], f32, kind="ExternalOutput").ap()

    Alu = mybir.AluOpType
    Act = mybir.ActivationFunctionType
    from concourse.bass_isa import ReduceOp

    FB = RPP * E           # free bytes per big tile row-group
    with tile.TileContext(nc) as tc, ExitStack() as ctx:
        consts = ctx.enter_context(tc.tile_pool(name="consts", bufs=1))
        mem_pool = ctx.enter_context(tc.tile_pool(name="memp", bufs=mem_bufs))
        om_pool = ctx.enter_context(tc.tile_pool(name="omp", bufs=om_bufs))
        scratch = ctx.enter_context(tc.tile_pool(name="scr", bufs=4))
        small = ctx.enter_context(tc.tile_pool(name="small", bufs=24))
        psum_pool = ctx.enter_context(
            tc.tile_pool(name="ps", bufs=4, space="PSUM"))
        wo_pool = ctx.enter_context(tc.tile_pool(name="wop", bufs=2))

        import concourse.bass as bass

        # consts on non-SP rings so the SP FIFO starts with mem loads
        lb_sb = consts.tile([P, BL, NCOL], f32)
        nc.scalar.dma_start(out=lb_sb, in_=lb)
        q_sb = consts.tile([P, BL, E], f32)
        nc.gpsimd.dma_start(
            out=q_sb,
            in_=bass.AP(tensor=q.tensor, offset=q.offset,
                        ap=[[0, P], [E, BL], [1, E]]))

        # ---- phase 1: scores + softmax for all batches (mem stream) ----
        p_ts = {}
        rinvs = {}
        for b in range(BL):
            scores = small.tile([P, NCOL], f32, tag="scores")
            for t in range(TB):
                m = mem_pool.tile([P, FB], f32, tag="m")
                nc.sync.dma_start(
                    out=m,
                    in_=mem[b, t * P * RPP:(t + 1) * P * RPP, :].rearrange(
                        "(p r) e -> p (r e)", p=P))
                for r in range(RPP):
                    scr = scratch.tile([P, E], f32, tag="scr")
                    nc.vector.affine_mul_reduce(
                        out=scr, accum_out=scores[:, RPP * t + r:RPP * t + r + 1],
                        in0=m[:, r * E:(r + 1) * E], in1=q_sb[:, b, :],
                        scale=1.0, bias=0.0)

            sm = small.tile([P, NCOL], f32, tag="sm")
            nc.vector.tensor_tensor(out=sm, in0=scores,
                                    in1=lb_sb[:, b, :], op=Alu.min)
            pmax = small.tile([P, 1], f32, tag="pmax")
            nc.vector.tensor_reduce(out=pmax, in_=sm,
                                    axis=mybir.AxisListType.X, op=Alu.max)
            gmax = small.tile([P, 1], f32, tag="gmax")
            nc.gpsimd.partition_all_reduce(gmax, pmax, P, ReduceOp.max)
            negmax = small.tile([P, 1], f32, tag="negmax")
            nc.vector.tensor_scalar_mul(negmax, gmax, -1.0)

            p_t = small.tile([P, NCOL], f32, tag="pt")
            sum_p = small.tile([P, 1], f32, tag="sump")
            nc.scalar.activation(out=p_t, in_=sm, func=Act.Exp,
                                 bias=negmax, accum_out=sum_p)
            gsum = small.tile([P, 1], f32, tag="gsum")
            nc.gpsimd.partition_all_reduce(gsum, sum_p, P, ReduceOp.add)
            rinv = small.tile([P, 1], f32, tag="rinv")
            nc.vector.reciprocal(rinv, gsum)
            att_t = small.tile([P, NCOL], f32, tag="att")
            nc.vector.tensor_scalar_mul(att_t, p_t, rinv)
            nc.gpsimd.dma_start(
                out=att[b].rearrange("(t p r) -> p t r", p=P, r=RPP),
                in_=att_t.rearrange("p (t r) -> p t r", r=RPP))
            p_ts[b] = p_t
            rinvs[b] = rinv

        # ---- phase 2: weighted output for all batches (om stream) ----
        # matmuls use the unnormalized exp(p_t); 1/sum is folded into the
        # PSUM->SBUF copy so they don't wait on the sum/reciprocal
        for b in range(BL):
            p_t = p_ts[b]
            ps0 = psum_pool.tile([1, NE_HALF], f32, tag="ps0")
            ps1 = psum_pool.tile([1, NE_HALF], f32, tag="ps1")
            for t in range(TB):
                o = om_pool.tile([P, FB], f32, tag="o")
                nc.scalar.dma_start(
                    out=o,
                    in_=om[b, t * P * RPP:(t + 1) * P * RPP, :].rearrange(
                        "(p r) e -> p (r e)", p=P))
                for r in range(RPP):
                    c = RPP * t + r
                    lhsT = p_t[:, c:c + 1]
                    first = (t == 0 and r == 0)
                    last = (t == TB - 1 and r == RPP - 1)
                    nc.tensor.matmul(ps0, lhsT=lhsT,
                                     rhs=o[:, r * E:r * E + NE_HALF],
                                     start=first, stop=last)
                    nc.tensor.matmul(ps1, lhsT=lhsT,
                                     rhs=o[:, r * E + NE_HALF:(r + 1) * E],
                                     start=first, stop=last)
            w = wo_pool.tile([1, E], f32, tag="w")
            nc.vector.tensor_scalar_mul(w[:, 0:NE_HALF], ps0, rinvs[b][0:1, :])
            nc.vector.tensor_scalar_mul(w[:, NE_HALF:E], ps1, rinvs[b][0:1, :])
            nc.scalar.dma_start(out=wo[b:b + 1, :], in_=w)

    nc.compile()
    return nc


def _get_nc():
    if "nc" not in _CACHE:
        _CACHE["nc"] = _build_nc()
    return _CACHE["nc"]


def kernel(memory, output_memory, query, memory_mask, maxlen):
    from concourse.bass_utils import run_bass_kernel_spmd

    memory = np.ascontiguousarray(np.asarray(memory), dtype=np.float32)
    output_memory = np.ascontiguousarray(np.asarray(output_memory),
                                         dtype=np.float32)
    query = np.ascontiguousarray(np.asarray(query), dtype=np.float32)
    memory_mask = np.asarray(memory_mask).astype(np.int64)
    maxlen = int(maxlen)
    assert memory.shape == (B, L, E) and query.shape == (B, E)
    assert maxlen == L

    kept = np.arange(L)[None, :] < memory_mask[:, None]        # [B, L]
    lb_full = np.where(kept, F32_MAX, F32_MIN).astype(np.float32)
    lb_dev = _lb_layout(lb_full)

    in_maps = [{
        "mem": memory[c * BL:(c + 1) * BL],
        "om": output_memory[c * BL:(c + 1) * BL],
        "q": query[c * BL:(c + 1) * BL],
        "lb": np.ascontiguousarray(lb_dev[c]),
    } for c in range(N_CORES)]

    res = run_bass_kernel_spmd(_get_nc(), in_maps,
                               core_ids=list(range(N_CORES)))
    att = np.concatenate([res.results[c]["att"] for c in range(N_CORES)], 0)
    wo = np.concatenate([res.results[c]["wo"] for c in range(N_CORES)], 0)
    return att.astype(np.float32), wo.astype(np.float32)
